# revision 56
# baseline (speedup 1.0000x reference)
"""DiffAttn (differential attention) Trainium2 Bass kernel — v3.

Self-contained: kernel(**inputs) takes the FULL unsharded inputs as numpy
arrays and returns the FULL output [2, 4096, 128] float32.

Sharding: 8 cores = (batch in {0,1}) x (query-block of 1024 rows). Each core
is FULLY independent (no collectives): it streams the whole batch's
activations (host-permuted to [p][c][q] with its own query block's columns
first), projects K and V for all 4096 keys locally, Q for its own 1024
queries, and runs both softmaxes + combined PV + RMSNorm for its queries.
Replicating the K/V projections beats the AllGather under the timeline model
(the collective is priced at ~67us, mostly unoverlappable) and keeps every
core's program trivially SPMD-identical.

fp8 projections (DoubleRow, 4x PE throughput): x and the weights are shipped
as e4m3 (weights pre-scaled by 64 so they sit in fp8's normal range; the 64^2
on the scores is folded into the softmax exp scale, exactly).  Q/K tolerate
plain fp8 (softmax output error ~0.1%).  V needs more precision, so V is
computed as  x8@wv8 + (x8@ws8 + r8@wv8)/32  where r8 = fp8(32*(x - x8)) and
ws8 = fp8(32*(wv' - wv8)) — a first-order residual expansion; the dropped
r*s cross term is ~0.1%.  The global 64x on V cancels in RMSNorm.

Attention layout: scores are computed TRANSPOSED ([sk, q], keys on
partitions) so exp(scores) feeds PV as the STATIONARY operand against a
MOVING V_aug = [V | ones] (129 cols); the ones column makes each PV
accumulation also produce the softmax denominator in column 128 of U — no
separate row-sum matmuls and no transposes in the post phase.  Key order is
attention-irrelevant, so K/V chunks process in host-permuted order.
Second-half heads live in base-partition-0 copies (kT2/qT2, moved with the
DVE stream-shuffle network): matmuls that share a PSUM bank must share a PE
row base (hardware tile-position rule), and the serial DMA queue is busy
streaming x when these copies are needed.

Post: attn = U1/s1 - lam*U2/s2; RMSNorm(attn) == RMSNorm(s1*attn), so
w := U1 - lam*(s1/s2)*U2 needs no 1/s1 division (torch eps is ~1e-5 of
mean(attn^2) here).  out = w * rsqrt(sum_h w^2) * [rmsw*(1-li)*sqrt(H)].
rsqrt runs on DVE (bit-trick + 2 Newton steps, ~5e-6 rel) so the ACT exp
stream never switches activation tables; the post runs on DVE/Pool per pass,
fully overlapped except the last pass's short chain.

PSUM (8 banks): s(2 bufs, 1 bank each) + 4 U slot banks + work(2 bufs) for
the next block's K/V accumulators.  Block 0's K/Q run c-inner in the four
U banks (idle until attention starts) so the PE ramps while x streams in.
"""

import math
import os
import sys
from contextlib import ExitStack

import numpy as np

for _p in ("/root/.axon_site/_ro/trn_rl_repo", "/opt/trn_rl_repo"):
    if os.path.isdir(_p) and _p not in sys.path:
        sys.path.append(_p)

import ml_dtypes  # noqa: E402

import concourse.bass as bass  # noqa: E402
import concourse.mybir as mybir  # noqa: E402
import concourse.tile as tile  # noqa: E402
from concourse import bacc, bass_utils  # noqa: E402

B, S, D, H = 2, 4096, 2048, 128
H2 = H // 2  # 64
P = 128
NCORES = 8
QSHARD = 1024  # q rows per core
DCH = D // P  # 16 d-chunks
NKCH = S // P  # 32 key chunks of 128
NBLK, BLKW = 4, 1024  # key blocks (1024 keys each)
NPASS, PW = 4, 256  # query passes of 256 columns (U psum = 4 banks)
WSCALE = 64.0  # host-side weight prescale (exact power of 2)

LAMBDA_INIT = 0.8 - 0.6 * math.exp(-0.3 * 12)
SCALE = 1.0 / math.sqrt(H2)
SCALE_EXP = SCALE / (WSCALE * WSCALE)  # exp input is (64q).(64k)

F32 = mybir.dt.float32
BF16 = mybir.dt.bfloat16
F8 = mybir.dt.float8e4
I32 = mybir.dt.int32

AF = mybir.ActivationFunctionType
OP = mybir.AluOpType
DR = mybir.MatmulPerfMode.DoubleRow


def _emit(ctx: ExitStack, tc: "tile.TileContext", lam: float):  # noqa: C901
    nc = tc.nc

    x8_d = nc.dram_tensor("x8", (P, DCH, S), F8, kind="ExternalInput").ap()
    r8_d = nc.dram_tensor("r8", (P, DCH, S), F8, kind="ExternalInput").ap()
    wpk_d = nc.dram_tensor("wpk", (P, 6, DCH, H), F8, kind="ExternalInput").ap()
    rmsw = nc.dram_tensor("rmsw", (H,), F32, kind="ExternalInput").ap()
    out_d = nc.dram_tensor("out", (QSHARD, H), F32, kind="ExternalOutput").ap()

    # ---- persistent SBUF ----
    consts = ctx.enter_context(tc.tile_pool(name="consts", bufs=1))
    persist = ctx.enter_context(tc.tile_pool(name="persist", bufs=1))

    wpk_sb = consts.tile([P, 6, DCH, H], F8)
    wk_sb, wq_sb, wks_sb, wqs_sb, wv_sb, ws_sb = (
        wpk_sb[:, i] for i in range(6)
    )
    rmsw_bc = consts.tile([P, H], F32)

    kT_sb = persist.tile([P, S], BF16)  # [h, key]
    qT_sb = persist.tile([P, QSHARD], BF16)  # [h, q]
    # second-half heads relocated to base partition 0 (PE tile-position rule)
    kT2_sb = persist.tile([64, S], BF16)
    qT2_sb = persist.tile([64, QSHARD], BF16)
    v_sb = persist.tile([P, NKCH, H + 1], BF16)  # [sk%128, chunk, h|1]
    u_acc = persist.tile([P, NPASS, 2, 2, H + 1], F32)  # [q, pass, hf, j2, h|s]
    w_all = persist.tile([P, 2 * NPASS, H], F32)  # [q, subtile, h]
    rsq_all = persist.tile([P, 2 * NPASS], F32)
    rr_all = persist.tile([P, 2 * NPASS], F32)
    rr_i = persist.tile([P, 2 * NPASS], I32)
    o_all = persist.tile([P, 2 * NPASS, H], F32)

    xpool = ctx.enter_context(tc.tile_pool(name="xstream", bufs=1))
    epool = ctx.enter_context(tc.tile_pool(name="epool", bufs=10))
    small = ctx.enter_context(tc.tile_pool(name="small", bufs=4))

    # ---- const DMAs; ones column of V_aug.  Weights ship as ONE packed
    # tensor (per-partition rows are 12KB contiguous: no small-descriptor
    # penalty); the c0-1 slice of the four k/q weights goes first ----
    nc.sync.dma_start(out=wpk_sb[:, 0:4, 0:2, :], in_=wpk_d[:, 0:4, 0:2, :])
    nc.vector.memset(v_sb[:, :, H : H + 1], 1.0)

    x_tiles, r_tiles = [], []
    for b in range(NBLK):
        x_tiles.append(
            xpool.tile([P, DCH, BLKW], F8, tag=f"x{b % 3}", bufs=1, name=f"xt{b}")
        )
        r_tiles.append(
            xpool.tile([P, DCH, BLKW], F8, tag=f"r{b % 3}", bufs=1, name=f"rt{b}")
        )
    # block-0 x8/r8 interleaved in c-pair slabs: the residual projections
    # pipeline with the DMA instead of waiting for the whole r8 block
    for cp in range(DCH // 2):
        nc.sync.dma_start(
            out=x_tiles[0][:, 2 * cp : 2 * cp + 2, :],
            in_=x8_d[:, 2 * cp : 2 * cp + 2, 0:BLKW],
        )
        if cp == 0:
            nc.sync.dma_start(
                out=wpk_sb[:, 0:4, 2:DCH, :], in_=wpk_d[:, 0:4, 2:DCH, :]
            )
        if cp == 1:
            nc.sync.dma_start(out=wpk_sb[:, 4:6], in_=wpk_d[:, 4:6])
        if cp == 2:
            nc.sync.dma_start(
                out=rmsw_bc,
                in_=bass.AP(tensor=rmsw.tensor, offset=0, ap=[[0, P], [1, H]]),
            )
        nc.sync.dma_start(
            out=r_tiles[0][:, 2 * cp : 2 * cp + 2, :],
            in_=r8_d[:, 2 * cp : 2 * cp + 2, 0:BLKW],
        )

    def x_slabs(b):
        for h2 in range(2):
            nc.sync.dma_start(
                out=x_tiles[b][:, h2 * 8 : (h2 + 1) * 8, :],
                in_=x8_d[:, h2 * 8 : (h2 + 1) * 8, b * BLKW : (b + 1) * BLKW],
            )
            nc.sync.dma_start(
                out=r_tiles[b][:, h2 * 8 : (h2 + 1) * 8, :],
                in_=r8_d[:, h2 * 8 : (h2 + 1) * 8, b * BLKW : (b + 1) * BLKW],
            )

    x_slabs(1)

    psum = ctx.enter_context(tc.tile_pool(name="psum", space="PSUM", bufs=1))

    def proj_copy_k(accm, accr, b, g):
        # hw: ALU ops may read only ONE input from PSUM -> copy main first,
        # then accumulate the scaled residual in place.  High priority: this
        # DVE chain gates the next block's entire exp stream.
        sl = slice(b * BLKW + g * 512, b * BLKW + (g + 1) * 512)
        with tc.high_priority(offset=1500):
            nc.vector.tensor_copy(kT_sb[:, sl], accm)
            nc.vector.scalar_tensor_tensor(
                kT_sb[:, sl], accr, 1.0 / 32.0, kT_sb[:, sl],
                op0=OP.mult, op1=OP.add,
            )
        # second-half heads to base partition 0 via the DVE shuffle network
        # (no DMA: the serial DMA queue is full of x slabs at this point)
        nc.vector.stream_shuffle(kT2_sb[:, sl], kT_sb[64:128, sl], list(range(32)))

    def kq_mm(acc, w_t, xt, cp, g, st, sp):
        nc.tensor.matmul(
            acc,
            w_t[:, 2 * cp : 2 * cp + 2, :],
            xt[:, 2 * cp : 2 * cp + 2, g * 512 : (g + 1) * 512],
            start=st,
            stop=sp,
            perf_mode=DR,
        )

    # ---- block 0: K and Q projections c-inner in the four (idle) U banks,
    # pipelined with the x8 c-pair DMAs so the PE starts early and ramps ----
    kaccm = [
        psum.tile([P, 512], F32, tag=f"u{g}", bufs=1, name=f"kaccm{g}")
        for g in range(2)
    ]
    qaccm = [
        psum.tile([P, 512], F32, tag=f"u{2 + g}", bufs=1, name=f"qaccm{g}")
        for g in range(2)
    ]
    kaccr = [
        psum.tile([P, 512], F32, tag="s", bufs=2, name=f"kaccr{g}")
        for g in range(2)
    ]
    qaccr = [
        psum.tile([P, 512], F32, tag="work", bufs=2, name=f"qaccr{g}")
        for g in range(2)
    ]
    # per c-pair: main, x8*ws residual, r8*w residual — everything only needs
    # that c-pair of x8/r8, so the whole phase is DMA-paced
    for cp in range(DCH // 2):
        st, sp = cp == 0, cp == DCH // 2 - 1
        for g in range(2):
            kq_mm(kaccm[g], wk_sb, x_tiles[0], cp, g, st, sp)
            kq_mm(qaccm[g], wq_sb, x_tiles[0], cp, g, st, sp)
            kq_mm(kaccr[g], wks_sb, x_tiles[0], cp, g, st, False)
            kq_mm(qaccr[g], wqs_sb, x_tiles[0], cp, g, st, False)
            kq_mm(kaccr[g], wk_sb, r_tiles[0], cp, g, False, sp)
            kq_mm(qaccr[g], wq_sb, r_tiles[0], cp, g, False, sp)
    for g in range(2):
        proj_copy_k(kaccm[g], kaccr[g], 0, g)
        qsl = slice(g * 512, (g + 1) * 512)
        nc.scalar.copy(qT_sb[:, qsl], qaccm[g])
        nc.vector.scalar_tensor_tensor(
            qT_sb[:, qsl], qaccr[g], 1.0 / 32.0, qT_sb[:, qsl],
            op0=OP.mult, op1=OP.add,
        )
        nc.vector.stream_shuffle(
            qT2_sb[:, g * 512 : (g + 1) * 512],
            qT_sb[64:128, g * 512 : (g + 1) * 512],
            list(range(32)),
        )
    x_slabs(2)
    x_slabs(3)

    def kproj_gen(b):
        """K projection for block b (fp8 main + residual, two work banks),
        sliced: yields between groups so emission interleaves with the
        attention chunk stream (the PE queue drains in order)."""
        for g in range(2):
            accm = psum.tile([P, 512], F32, tag="work", bufs=2, name=f"km{b}{g}")
            accr = psum.tile([P, 512], F32, tag="work", bufs=2, name=f"kr{b}{g}")
            for cp in range(DCH // 2):
                kq_mm(accm, wk_sb, x_tiles[b], cp, g, cp == 0, cp == DCH // 2 - 1)
            yield
            for cp in range(DCH // 2):
                kq_mm(accr, wks_sb, x_tiles[b], cp, g, cp == 0, False)
            for cp in range(DCH // 2):
                kq_mm(accr, wk_sb, r_tiles[b], cp, g, False, cp == DCH // 2 - 1)
            proj_copy_k(accm, accr, b, g)
            yield

    def vproj_gen(b):
        """V projection for block b: fp8 main term plus (1/32)-scaled
        residual terms, two work banks, j-outer (sequential groups/bank);
        yields between j-slices for emission interleaving."""
        for g in range(2):
            vmain = psum.tile([P, 4, P], F32, tag="work", bufs=2, name=f"vm{b}{g}")
            vres = psum.tile([P, 4, P], F32, tag="work", bufs=2, name=f"vr{b}{g}")
            for j in range(4):
                ksl = slice(g * 512 + j * P, g * 512 + (j + 1) * P)
                for cp in range(DCH // 2):
                    csl = slice(2 * cp, 2 * cp + 2)
                    nc.tensor.matmul(
                        vmain[:, j, :],
                        x_tiles[b][:, csl, ksl],
                        wv_sb[:, csl, :],
                        start=(cp == 0),
                        stop=(cp == DCH // 2 - 1),
                        perf_mode=DR,
                    )
                for cp in range(DCH // 2):
                    csl = slice(2 * cp, 2 * cp + 2)
                    nc.tensor.matmul(
                        vres[:, j, :],
                        x_tiles[b][:, csl, ksl],
                        ws_sb[:, csl, :],
                        start=(cp == 0),
                        stop=False,
                        perf_mode=DR,
                    )
                for cp in range(DCH // 2):
                    csl = slice(2 * cp, 2 * cp + 2)
                    nc.tensor.matmul(
                        vres[:, j, :],
                        r_tiles[b][:, csl, ksl],
                        wv_sb[:, csl, :],
                        start=False,
                        stop=(cp == DCH // 2 - 1),
                        perf_mode=DR,
                    )
                yield
            vsl = v_sb[:, b * 8 + g * 4 : b * 8 + (g + 1) * 4, 0:H]
            nc.vector.tensor_copy(vsl, vmain)
            nc.vector.scalar_tensor_tensor(
                vsl, vres, 1.0 / 32.0, vsl, op0=OP.mult, op1=OP.add
            )

    def run_all(gen):
        for _ in gen:
            pass

    stage = int(os.environ.get("KSTAGE", "3"))
    if stage <= 1:
        run_all(vproj_gen(0))
        for b in range(1, NBLK):
            run_all(kproj_gen(b))
            run_all(vproj_gen(b))
        o_dbg = small.tile([P, H], F32, tag="odbg")
        nc.vector.tensor_copy(o_dbg, kT_sb[:, 0:H])
        for t in range(2 * NPASS):
            nc.sync.dma_start(out=out_d[t * P : (t + 1) * P, :], in_=o_dbg)
        return

    def attend_block_pass(b, p, fillers=iter(())):
        """Pass p (256 q cols), all 8 key chunks of block b -> U psum, then
        drain-add into u_acc.  A filler slice of the next block's projection
        work is emitted mid-pass and at pass end."""
        q0 = p * PW
        u_ps = [
            psum.tile([P, 512], F32, tag=f"u{s}", bufs=1, name=f"u{b}_{p}_{s}")
            for s in range(4)
        ]
        for cl in range(8):
            ch = b * 8 + cl
            s_ps = psum.tile([P, 512], F32, tag="s", bufs=2)
            with tc.high_priority(offset=5000):
                nc.tensor.matmul(
                    s_ps[:, 0:PW],
                    kT_sb[0:H2, ch * P : (ch + 1) * P],
                    qT_sb[0:H2, q0 : q0 + PW],
                )
                nc.tensor.matmul(
                    s_ps[:, PW : 2 * PW],
                    kT2_sb[:, ch * P : (ch + 1) * P],
                    qT2_sb[:, q0 : q0 + PW],
                )
                e_sb = epool.tile([P, 512], BF16, tag="e")
                nc.scalar.activation(e_sb, s_ps, AF.Exp, scale=SCALE_EXP)
            for hf in range(2):
                for j2 in range(2):
                    nc.tensor.matmul(
                        u_ps[hf * 2 + j2][:, 0 : H + 1],
                        e_sb[:, hf * PW + j2 * P : hf * PW + (j2 + 1) * P],
                        v_sb[:, ch, :],
                        start=(cl == 0),
                        stop=(cl == 7),
                    )
        with tc.high_priority(offset=600):
            for slot in range(4):
                hf, j2 = divmod(slot, 2)
                dst = u_acc[:, p, hf, j2, :]
                src = u_ps[slot][:, 0 : H + 1]
                if b == 0:
                    nc.vector.tensor_copy(dst, src)
                else:
                    nc.vector.tensor_tensor(dst, src, dst, op=OP.add)

    def post_pass(p):
        """Normalize/combine for pass p's two q-subtiles (DVE + Pool only)."""
        for j2 in range(2):
            t = p * 2 + j2
            s1 = u_acc[:, p, 0, j2, H : H + 1]
            s2 = u_acc[:, p, 1, j2, H : H + 1]
            r2 = small.tile([P, 1], F32, tag="r2")
            nc.vector.reciprocal(r2, s2)
            al = small.tile([P, 1], F32, tag="al")
            nc.vector.scalar_tensor_tensor(
                al, s1, -lam, r2, op0=OP.mult, op1=OP.mult
            )
            nc.vector.scalar_tensor_tensor(
                w_all[:, t, :],
                u_acc[:, p, 1, j2, 0:H],
                al,
                u_acc[:, p, 0, j2, 0:H],
                op0=OP.mult,
                op1=OP.add,
            )
            sq = small.tile([P, H], F32, tag="sq")
            nc.gpsimd.tensor_tensor(sq, w_all[:, t, :], w_all[:, t, :], op=OP.mult)
            nc.vector.tensor_reduce(
                rsq_all[:, t : t + 1], sq, axis=mybir.AxisListType.X, op=OP.add
            )
        # rr = rsqrt(ssq) on DVE (bit-trick + 2 Newton steps, ~5e-6 rel):
        # keeps the ACT exp stream free of Sqrt table switches
        t0 = p * 2
        ss = rsq_all[:, t0 : t0 + 2]
        yi = rr_i[:, t0 : t0 + 2]
        y = yi.bitcast(F32)
        nc.vector.tensor_scalar(
            yi, ss.bitcast(I32), 1, None, op0=OP.arith_shift_right
        )
        nc.vector.tensor_scalar(yi, yi, -1, 0x5F3759DF, op0=OP.mult, op1=OP.add)
        nt = small.tile([P, 2], F32, tag="nt")
        for _ in range(2):
            nc.vector.tensor_tensor(nt, ss, y, op=OP.mult)
            nc.vector.tensor_tensor(nt, nt, y, op=OP.mult)
            nc.vector.tensor_scalar(nt, nt, -0.5, 1.5, op0=OP.mult, op1=OP.add)
            nc.vector.tensor_tensor(y, y, nt, op=OP.mult)
        nc.vector.tensor_copy(rr_all[:, t0 : t0 + 2], y)
        for t in (t0, t0 + 1):
            nc.vector.scalar_tensor_tensor(
                o_all[:, t, :], w_all[:, t, :], rr_all[:, t : t + 1], rmsw_bc,
                op0=OP.mult, op1=OP.mult,
            )
        nc.scalar.dma_start(
            out=out_d[t0 * P : (t0 + 2) * P, :].rearrange("(t p) h -> p t h", p=P),
            in_=o_all[:, t0 : t0 + 2, :],
        )

    # ---- main schedule: per block: 4 passes, with the next block's K/V
    # projection work sliced into small fillers emitted between chunks so
    # the PE queue interleaves them with the score/exp stream ----
    run_all(vproj_gen(0))
    for b in range(NBLK):
        for p in range(NPASS):
            attend_block_pass(b, p)
            if b < NBLK - 1:
                if p == 1:
                    run_all(kproj_gen(b + 1))
                elif p == 2:
                    vg = vproj_gen(b + 1)
                    for _ in range(4):
                        next(vg, None)
                elif p == 3:
                    for _ in vg:
                        pass
            if b == NBLK - 1 and stage >= 3:
                with tc.high_priority(offset=900):
                    post_pass(p)


def build(lam: float):
    from concourse._compat import axon_active

    nc = bacc.Bacc(
        "TRN2",
        target_bir_lowering=False,
        debug=not axon_active(),
        num_devices=NCORES,
    )
    with tile.TileContext(nc) as tc:
        with ExitStack() as ctx:
            _emit(ctx, tc, lam)
    nc.compile()
    return nc


def make_in_maps(x, Wq, Wk, Wv, rms_weight):
    f8 = ml_dtypes.float8_e4m3
    x = np.asarray(x, dtype=np.float32)
    xT = np.ascontiguousarray(x.transpose(0, 2, 1))  # [B, D, S]
    xP = xT.reshape(B, DCH, P, S).transpose(0, 2, 1, 3)  # [B, p, c, S]

    def wsplit(W):
        # w'[p, c, h] = 64*W[h, c*128+p]; fp8 main + fp8 32x-scaled residual
        WT = np.asarray(W, np.float32).T * WSCALE  # [D, H]
        wp = np.ascontiguousarray(WT.reshape(DCH, P, H).transpose(1, 0, 2))
        w8 = wp.astype(f8)
        ws8 = ((wp - w8.astype(np.float32)) * 32.0).astype(f8)
        return np.ascontiguousarray(w8), np.ascontiguousarray(ws8)

    wq8, wqs8 = wsplit(Wq)
    wk8, wks8 = wsplit(Wk)
    wv8, ws8 = wsplit(Wv)
    wpk = np.ascontiguousarray(
        np.stack([wk8, wq8, wks8, wqs8, wv8, ws8], axis=1)
    )
    c_ = 1.0 - LAMBDA_INIT
    rw = np.ascontiguousarray(
        np.asarray(rms_weight, np.float32) * c_ * math.sqrt(H)
    ).astype(np.float32)

    in_maps = []
    for core in range(NCORES):
        b, qb = divmod(core, NCORES // B)
        # own query block's columns first; key order is attention-irrelevant
        cols = np.r_[qb * QSHARD : (qb + 1) * QSHARD,
                     0 : qb * QSHARD, (qb + 1) * QSHARD : S]
        xc = xP[b][:, :, cols]
        x8 = np.ascontiguousarray(xc.astype(f8))
        r8 = np.ascontiguousarray(
            ((xc - x8.astype(np.float32)) * 32.0).astype(f8)
        )
        in_maps.append(
            {"x8": x8, "r8": r8, "wpk": wpk, "rmsw": rw}
        )
    # e4m3 and e4m3fn are byte-identical over our value range (<224); jax's
    # transfer path only accepts the fn variant, the device reads raw bytes
    fn = ml_dtypes.float8_e4m3fn
    for m in in_maps:
        for k, v in m.items():
            if v.dtype == f8:
                m[k] = v.view(fn)
    return in_maps


def kernel(x, Wq, Wk, Wv, lambda_q1, lambda_q2, lambda_k1, lambda_k2, rms_weight):
    lq1 = np.asarray(lambda_q1, np.float32)
    lq2 = np.asarray(lambda_q2, np.float32)
    lk1 = np.asarray(lambda_k1, np.float32)
    lk2 = np.asarray(lambda_k2, np.float32)
    lam = float(
        np.exp(np.dot(lq1, lk1)) - np.exp(np.dot(lq2, lk2)) + LAMBDA_INIT
    )
    nc = build(lam)
    in_maps = make_in_maps(x, Wq, Wk, Wv, rms_weight)
    res = bass_utils.run_bass_kernel_spmd(nc, in_maps, core_ids=list(range(NCORES)))
    out = np.empty((B, S, H), np.float32)
    for core in range(NCORES):
        b, qb = divmod(core, NCORES // B)
        out[b, qb * QSHARD : (qb + 1) * QSHARD] = res.results[core]["out"]
    return out


# revision 62
# speedup vs baseline: 1.0154x; 1.0154x over previous
"""DiffAttn (differential attention) Trainium2 Bass kernel — v3.

Self-contained: kernel(**inputs) takes the FULL unsharded inputs as numpy
arrays and returns the FULL output [2, 4096, 128] float32.

Sharding: 8 cores = (batch in {0,1}) x (query-block of 1024 rows). Each core
is FULLY independent (no collectives): it streams the whole batch's
activations (host-permuted to [p][c][q] with its own query block's columns
first), projects K and V for all 4096 keys locally, Q for its own 1024
queries, and runs both softmaxes + combined PV + RMSNorm for its queries.
Replicating the K/V projections beats the AllGather under the timeline model
(the collective is priced at ~67us, mostly unoverlappable) and keeps every
core's program trivially SPMD-identical.

fp8 projections (DoubleRow, 4x PE throughput): x and the weights are shipped
as e4m3 (weights pre-scaled by 64 so they sit in fp8's normal range; the 64^2
on the scores is folded into the softmax exp scale, exactly).  Q/K tolerate
plain fp8 (softmax output error ~0.1%).  V needs more precision, so V is
computed as  x8@wv8 + (x8@ws8 + r8@wv8)/32  where r8 = fp8(32*(x - x8)) and
ws8 = fp8(32*(wv' - wv8)) — a first-order residual expansion; the dropped
r*s cross term is ~0.1%.  The global 64x on V cancels in RMSNorm.

Attention layout: scores are computed TRANSPOSED ([sk, q], keys on
partitions) so exp(scores) feeds PV as the STATIONARY operand against a
MOVING V_aug = [V | ones] (129 cols); the ones column makes each PV
accumulation also produce the softmax denominator in column 128 of U — no
separate row-sum matmuls and no transposes in the post phase.  Key order is
attention-irrelevant, so K/V chunks process in host-permuted order.
Second-half heads live in base-partition-0 copies (kT2/qT2, moved with the
DVE stream-shuffle network): matmuls that share a PSUM bank must share a PE
row base (hardware tile-position rule), and the serial DMA queue is busy
streaming x when these copies are needed.

Post: attn = U1/s1 - lam*U2/s2; RMSNorm(attn) == RMSNorm(s1*attn), so
w := U1 - lam*(s1/s2)*U2 needs no 1/s1 division (torch eps is ~1e-5 of
mean(attn^2) here).  out = w * rsqrt(sum_h w^2) * [rmsw*(1-li)*sqrt(H)].
rsqrt runs on DVE (bit-trick + 2 Newton steps, ~5e-6 rel) so the ACT exp
stream never switches activation tables; the post runs on DVE/Pool per pass,
fully overlapped except the last pass's short chain.

PSUM (8 banks): s(2 bufs, 1 bank each) + 4 U slot banks + work(2 bufs) for
the next block's K/V accumulators.  Block 0's K/Q run c-inner in the four
U banks (idle until attention starts) so the PE ramps while x streams in.
"""

import math
import os
import sys
from contextlib import ExitStack

import numpy as np

for _p in ("/root/.axon_site/_ro/trn_rl_repo", "/opt/trn_rl_repo"):
    if os.path.isdir(_p) and _p not in sys.path:
        sys.path.append(_p)

import ml_dtypes  # noqa: E402

import concourse.bass as bass  # noqa: E402
import concourse.mybir as mybir  # noqa: E402
import concourse.tile as tile  # noqa: E402
from concourse import bacc, bass_utils  # noqa: E402

B, S, D, H = 2, 4096, 2048, 128
H2 = H // 2  # 64
P = 128
NCORES = 8
QSHARD = 1024  # q rows per core
DCH = D // P  # 16 d-chunks
NKCH = S // P  # 32 key chunks of 128
NBLK, BLKW = 4, 1024  # key blocks (1024 keys each)
NPASS, PW = 4, 256  # query passes of 256 columns (U psum = 4 banks)
WSCALE = 64.0  # host-side weight prescale (exact power of 2)

LAMBDA_INIT = 0.8 - 0.6 * math.exp(-0.3 * 12)
SCALE = 1.0 / math.sqrt(H2)
SCALE_EXP = SCALE / (WSCALE * WSCALE)  # exp input is (64q).(64k)

F32 = mybir.dt.float32
BF16 = mybir.dt.bfloat16
F8 = mybir.dt.float8e4
I32 = mybir.dt.int32

AF = mybir.ActivationFunctionType
OP = mybir.AluOpType
DR = mybir.MatmulPerfMode.DoubleRow


def _emit(ctx: ExitStack, tc: "tile.TileContext", lam: float):  # noqa: C901
    nc = tc.nc

    x8_d = nc.dram_tensor("x8", (P, DCH, S), F8, kind="ExternalInput").ap()
    r8_d = nc.dram_tensor("r8", (P, DCH, S), F8, kind="ExternalInput").ap()
    wpk_d = nc.dram_tensor("wpk", (P, 6, DCH, H), F8, kind="ExternalInput").ap()
    rmsw = nc.dram_tensor("rmsw", (H,), F32, kind="ExternalInput").ap()
    out_d = nc.dram_tensor("out", (QSHARD, H), F32, kind="ExternalOutput").ap()

    # ---- persistent SBUF ----
    consts = ctx.enter_context(tc.tile_pool(name="consts", bufs=1))
    persist = ctx.enter_context(tc.tile_pool(name="persist", bufs=1))

    wpk_sb = consts.tile([P, 6, DCH, H], F8)
    wk_sb, wq_sb, wks_sb, wqs_sb, wv_sb, ws_sb = (
        wpk_sb[:, i] for i in range(6)
    )
    rmsw_bc = consts.tile([P, H], F32)

    kT_sb = persist.tile([P, S], BF16)  # [h, key]
    qT_sb = persist.tile([P, QSHARD], BF16)  # [h, q]
    # second-half heads relocated to base partition 0 (PE tile-position rule)
    kT2_sb = persist.tile([64, S], BF16)
    qT2_sb = persist.tile([64, QSHARD], BF16)
    v_sb = persist.tile([P, NKCH, H + 1], BF16)  # [sk%128, chunk, h|1]
    u_acc = persist.tile([P, NPASS, 2, 2, H + 1], F32)  # [q, pass, hf, j2, h|s]
    w_all = persist.tile([P, 2 * NPASS, H], F32)  # [q, subtile, h]
    rsq_all = persist.tile([P, 2 * NPASS], F32)
    rr_all = persist.tile([P, 2 * NPASS], F32)
    rr_i = persist.tile([P, 2 * NPASS], I32)
    o_all = persist.tile([P, 2 * NPASS, H], F32)

    xpool = ctx.enter_context(tc.tile_pool(name="xstream", bufs=1))
    epool = ctx.enter_context(tc.tile_pool(name="epool", bufs=52))
    small = ctx.enter_context(tc.tile_pool(name="small", bufs=4))

    # ---- const DMAs; ones column of V_aug.  Weights ship as ONE packed
    # tensor (per-partition rows are 12KB contiguous: no small-descriptor
    # penalty); the c0-1 slice of the four k/q weights goes first ----
    nc.sync.dma_start(out=wpk_sb[:, 0:4, 0:2, :], in_=wpk_d[:, 0:4, 0:2, :])
    nc.vector.memset(v_sb[:, :, H : H + 1], 1.0)

    x_tiles, r_tiles = [], []
    for b in range(NBLK):
        x_tiles.append(
            xpool.tile([P, DCH, BLKW], F8, tag=f"x{b % 3}", bufs=1, name=f"xt{b}")
        )
        r_tiles.append(
            xpool.tile([P, DCH, BLKW], F8, tag=f"r{b % 3}", bufs=1, name=f"rt{b}")
        )
    # block-0 x8/r8 interleaved in c-pair slabs: the residual projections
    # pipeline with the DMA instead of waiting for the whole r8 block
    for cp in range(DCH // 2):
        nc.sync.dma_start(
            out=x_tiles[0][:, 2 * cp : 2 * cp + 2, :],
            in_=x8_d[:, 2 * cp : 2 * cp + 2, 0:BLKW],
        )
        if cp == 0:
            nc.sync.dma_start(
                out=wpk_sb[:, 0:4, 2:DCH, :], in_=wpk_d[:, 0:4, 2:DCH, :]
            )
        if cp == 1:
            nc.sync.dma_start(out=wpk_sb[:, 4:6], in_=wpk_d[:, 4:6])
        if cp == 2:
            nc.sync.dma_start(
                out=rmsw_bc,
                in_=bass.AP(tensor=rmsw.tensor, offset=0, ap=[[0, P], [1, H]]),
            )
        nc.sync.dma_start(
            out=r_tiles[0][:, 2 * cp : 2 * cp + 2, :],
            in_=r8_d[:, 2 * cp : 2 * cp + 2, 0:BLKW],
        )

    def x_slabs(b):
        for h2 in range(2):
            nc.sync.dma_start(
                out=x_tiles[b][:, h2 * 8 : (h2 + 1) * 8, :],
                in_=x8_d[:, h2 * 8 : (h2 + 1) * 8, b * BLKW : (b + 1) * BLKW],
            )
            nc.sync.dma_start(
                out=r_tiles[b][:, h2 * 8 : (h2 + 1) * 8, :],
                in_=r8_d[:, h2 * 8 : (h2 + 1) * 8, b * BLKW : (b + 1) * BLKW],
            )

    x_slabs(1)

    psum = ctx.enter_context(tc.tile_pool(name="psum", space="PSUM", bufs=1))

    def proj_copy_k(accm, accr, b, g):
        # hw: ALU ops may read only ONE input from PSUM -> copy main first,
        # then accumulate the scaled residual in place.  High priority: this
        # DVE chain gates the next block's entire exp stream.
        sl = slice(b * BLKW + g * 512, b * BLKW + (g + 1) * 512)
        with tc.high_priority(offset=1500):
            nc.vector.tensor_copy(kT_sb[:, sl], accm)
            nc.vector.scalar_tensor_tensor(
                kT_sb[:, sl], accr, 1.0 / 32.0, kT_sb[:, sl],
                op0=OP.mult, op1=OP.add,
            )
        # second-half heads to base partition 0 via the DVE shuffle network
        # (no DMA: the serial DMA queue is full of x slabs at this point)
        nc.vector.stream_shuffle(kT2_sb[:, sl], kT_sb[64:128, sl], list(range(32)))

    def kq_mm(acc, w_t, xt, cp, g, st, sp):
        nc.tensor.matmul(
            acc,
            w_t[:, 2 * cp : 2 * cp + 2, :],
            xt[:, 2 * cp : 2 * cp + 2, g * 512 : (g + 1) * 512],
            start=st,
            stop=sp,
            perf_mode=DR,
        )

    # ---- block 0: K and Q projections c-inner in the four (idle) U banks,
    # pipelined with the x8 c-pair DMAs so the PE starts early and ramps ----
    kaccm = [
        psum.tile([P, 512], F32, tag=f"u{g}", bufs=1, name=f"kaccm{g}")
        for g in range(2)
    ]
    qaccm = [
        psum.tile([P, 512], F32, tag=f"u{2 + g}", bufs=1, name=f"qaccm{g}")
        for g in range(2)
    ]
    kaccr = [
        psum.tile([P, 512], F32, tag="s", bufs=2, name=f"kaccr{g}")
        for g in range(2)
    ]
    qaccr = [
        psum.tile([P, 512], F32, tag="work", bufs=2, name=f"qaccr{g}")
        for g in range(2)
    ]
    # per c-pair: main, x8*ws residual, r8*w residual — everything only needs
    # that c-pair of x8/r8, so the whole phase is DMA-paced
    for cp in range(DCH // 2):
        st, sp = cp == 0, cp == DCH // 2 - 1
        for g in range(2):
            kq_mm(kaccm[g], wk_sb, x_tiles[0], cp, g, st, sp)
            kq_mm(qaccm[g], wq_sb, x_tiles[0], cp, g, st, sp)
            kq_mm(kaccr[g], wks_sb, x_tiles[0], cp, g, st, False)
            kq_mm(qaccr[g], wqs_sb, x_tiles[0], cp, g, st, False)
            kq_mm(kaccr[g], wk_sb, r_tiles[0], cp, g, False, sp)
            kq_mm(qaccr[g], wq_sb, r_tiles[0], cp, g, False, sp)
    for g in range(2):
        proj_copy_k(kaccm[g], kaccr[g], 0, g)
        qsl = slice(g * 512, (g + 1) * 512)
        nc.scalar.copy(qT_sb[:, qsl], qaccm[g])
        nc.vector.scalar_tensor_tensor(
            qT_sb[:, qsl], qaccr[g], 1.0 / 32.0, qT_sb[:, qsl],
            op0=OP.mult, op1=OP.add,
        )
        nc.vector.stream_shuffle(
            qT2_sb[:, g * 512 : (g + 1) * 512],
            qT_sb[64:128, g * 512 : (g + 1) * 512],
            list(range(32)),
        )
    x_slabs(2)
    x_slabs(3)

    def kproj_gen(b):
        """K projection for block b (fp8 main + residual, two work banks),
        sliced: yields between groups so emission interleaves with the
        attention chunk stream (the PE queue drains in order)."""
        for g in range(2):
            accm = psum.tile([P, 512], F32, tag="work", bufs=2, name=f"km{b}{g}")
            accr = psum.tile([P, 512], F32, tag="work", bufs=2, name=f"kr{b}{g}")
            for cp in range(DCH // 2):
                kq_mm(accm, wk_sb, x_tiles[b], cp, g, cp == 0, cp == DCH // 2 - 1)
            yield
            for cp in range(DCH // 2):
                kq_mm(accr, wks_sb, x_tiles[b], cp, g, cp == 0, False)
            for cp in range(DCH // 2):
                kq_mm(accr, wk_sb, r_tiles[b], cp, g, False, cp == DCH // 2 - 1)
            proj_copy_k(accm, accr, b, g)
            yield

    def vproj_gen(b):
        """V projection for block b: fp8 main term plus (1/32)-scaled
        residual terms, two work banks, j-outer (sequential groups/bank);
        yields between j-slices for emission interleaving."""
        for g in range(2):
            vmain = psum.tile([P, 4, P], F32, tag="work", bufs=2, name=f"vm{b}{g}")
            vres = psum.tile([P, 4, P], F32, tag="work", bufs=2, name=f"vr{b}{g}")
            for j in range(4):
                ksl = slice(g * 512 + j * P, g * 512 + (j + 1) * P)
                for cp in range(DCH // 2):
                    csl = slice(2 * cp, 2 * cp + 2)
                    nc.tensor.matmul(
                        vmain[:, j, :],
                        x_tiles[b][:, csl, ksl],
                        wv_sb[:, csl, :],
                        start=(cp == 0),
                        stop=(cp == DCH // 2 - 1),
                        perf_mode=DR,
                    )
                for cp in range(DCH // 2):
                    csl = slice(2 * cp, 2 * cp + 2)
                    nc.tensor.matmul(
                        vres[:, j, :],
                        x_tiles[b][:, csl, ksl],
                        ws_sb[:, csl, :],
                        start=(cp == 0),
                        stop=False,
                        perf_mode=DR,
                    )
                for cp in range(DCH // 2):
                    csl = slice(2 * cp, 2 * cp + 2)
                    nc.tensor.matmul(
                        vres[:, j, :],
                        r_tiles[b][:, csl, ksl],
                        wv_sb[:, csl, :],
                        start=False,
                        stop=(cp == DCH // 2 - 1),
                        perf_mode=DR,
                    )
                yield
            vsl = v_sb[:, b * 8 + g * 4 : b * 8 + (g + 1) * 4, 0:H]
            nc.vector.tensor_copy(vsl, vmain)
            nc.vector.scalar_tensor_tensor(
                vsl, vres, 1.0 / 32.0, vsl, op0=OP.mult, op1=OP.add
            )

    def run_all(gen):
        for _ in gen:
            pass

    stage = int(os.environ.get("KSTAGE", "3"))
    if stage <= 1:
        run_all(vproj_gen(0))
        for b in range(1, NBLK):
            run_all(kproj_gen(b))
            run_all(vproj_gen(b))
        o_dbg = small.tile([P, H], F32, tag="odbg")
        nc.vector.tensor_copy(o_dbg, kT_sb[:, 0:H])
        for t in range(2 * NPASS):
            nc.sync.dma_start(out=out_d[t * P : (t + 1) * P, :], in_=o_dbg)
        return

    def attend_block_pass(b, p, fillers=iter(())):
        """Pass p (256 q cols), all 8 key chunks of block b -> U psum, then
        drain-add into u_acc.  A filler slice of the next block's projection
        work is emitted mid-pass and at pass end."""
        q0 = p * PW
        u_ps = [
            psum.tile([P, 512], F32, tag=f"u{s}", bufs=1, name=f"u{b}_{p}_{s}")
            for s in range(4)
        ]
        for cl in range(8):
            ch = b * 8 + cl
            s_ps = psum.tile([P, 512], F32, tag="s", bufs=2)
            with tc.high_priority(offset=5000):
                nc.tensor.matmul(
                    s_ps[:, 0:PW],
                    kT_sb[0:H2, ch * P : (ch + 1) * P],
                    qT_sb[0:H2, q0 : q0 + PW],
                )
                nc.tensor.matmul(
                    s_ps[:, PW : 2 * PW],
                    kT2_sb[:, ch * P : (ch + 1) * P],
                    qT2_sb[:, q0 : q0 + PW],
                )
                e_sb = epool.tile([P, 512], BF16, tag="e")
                nc.scalar.activation(e_sb, s_ps, AF.Exp, scale=SCALE_EXP)
            for hf in range(2):
                for j2 in range(2):
                    nc.tensor.matmul(
                        u_ps[hf * 2 + j2][:, 0 : H + 1],
                        e_sb[:, hf * PW + j2 * P : hf * PW + (j2 + 1) * P],
                        v_sb[:, ch, :],
                        start=(cl == 0),
                        stop=(cl == 7),
                    )
        with tc.high_priority(offset=600):
            for slot in range(4):
                hf, j2 = divmod(slot, 2)
                dst = u_acc[:, p, hf, j2, :]
                src = u_ps[slot][:, 0 : H + 1]
                if b == 0:
                    nc.vector.tensor_copy(dst, src)
                else:
                    nc.vector.tensor_tensor(dst, src, dst, op=OP.add)

    def post_pass(p):
        """Normalize/combine for pass p's two q-subtiles (DVE + Pool only)."""
        for j2 in range(2):
            t = p * 2 + j2
            s1 = u_acc[:, p, 0, j2, H : H + 1]
            s2 = u_acc[:, p, 1, j2, H : H + 1]
            r2 = small.tile([P, 1], F32, tag="r2")
            nc.vector.reciprocal(r2, s2)
            al = small.tile([P, 1], F32, tag="al")
            nc.vector.scalar_tensor_tensor(
                al, s1, -lam, r2, op0=OP.mult, op1=OP.mult
            )
            nc.vector.scalar_tensor_tensor(
                w_all[:, t, :],
                u_acc[:, p, 1, j2, 0:H],
                al,
                u_acc[:, p, 0, j2, 0:H],
                op0=OP.mult,
                op1=OP.add,
            )
            sq = small.tile([P, H], F32, tag="sq")
            nc.gpsimd.tensor_tensor(sq, w_all[:, t, :], w_all[:, t, :], op=OP.mult)
            nc.vector.tensor_reduce(
                rsq_all[:, t : t + 1], sq, axis=mybir.AxisListType.X, op=OP.add
            )
        # rr = rsqrt(ssq) on DVE (bit-trick + 2 Newton steps, ~5e-6 rel):
        # keeps the ACT exp stream free of Sqrt table switches
        t0 = p * 2
        ss = rsq_all[:, t0 : t0 + 2]
        yi = rr_i[:, t0 : t0 + 2]
        y = yi.bitcast(F32)
        nc.vector.tensor_scalar(
            yi, ss.bitcast(I32), 1, None, op0=OP.arith_shift_right
        )
        nc.vector.tensor_scalar(yi, yi, -1, 0x5F3759DF, op0=OP.mult, op1=OP.add)
        nt = small.tile([P, 2], F32, tag="nt")
        for _ in range(2):
            nc.vector.tensor_tensor(nt, ss, y, op=OP.mult)
            nc.vector.tensor_tensor(nt, nt, y, op=OP.mult)
            nc.vector.tensor_scalar(nt, nt, -0.5, 1.5, op0=OP.mult, op1=OP.add)
            nc.vector.tensor_tensor(y, y, nt, op=OP.mult)
        nc.vector.tensor_copy(rr_all[:, t0 : t0 + 2], y)
        for t in (t0, t0 + 1):
            nc.vector.scalar_tensor_tensor(
                o_all[:, t, :], w_all[:, t, :], rr_all[:, t : t + 1], rmsw_bc,
                op0=OP.mult, op1=OP.mult,
            )
        nc.scalar.dma_start(
            out=out_d[t0 * P : (t0 + 2) * P, :].rearrange("(t p) h -> p t h", p=P),
            in_=o_all[:, t0 : t0 + 2, :],
        )

    # ---- main schedule: per block: 4 passes, with the next block's K/V
    # projection work sliced into small fillers emitted between chunks so
    # the PE queue interleaves them with the score/exp stream ----
    run_all(vproj_gen(0))
    for b in range(NBLK):
        for p in range(NPASS):
            attend_block_pass(b, p)
            if b < NBLK - 1:
                if p == 1:
                    run_all(kproj_gen(b + 1))
                elif p == 2:
                    vg = vproj_gen(b + 1)
                    for _ in range(4):
                        next(vg, None)
                elif p == 3:
                    for _ in vg:
                        pass
            if b == NBLK - 1 and stage >= 3:
                with tc.high_priority(offset=900):
                    post_pass(p)


def build(lam: float):
    from concourse._compat import axon_active

    nc = bacc.Bacc(
        "TRN2",
        target_bir_lowering=False,
        debug=not axon_active(),
        num_devices=NCORES,
    )
    with tile.TileContext(nc) as tc:
        with ExitStack() as ctx:
            _emit(ctx, tc, lam)
    nc.compile()
    return nc


def make_in_maps(x, Wq, Wk, Wv, rms_weight):
    f8 = ml_dtypes.float8_e4m3
    x = np.asarray(x, dtype=np.float32)
    xT = np.ascontiguousarray(x.transpose(0, 2, 1))  # [B, D, S]
    xP = xT.reshape(B, DCH, P, S).transpose(0, 2, 1, 3)  # [B, p, c, S]

    def wsplit(W):
        # w'[p, c, h] = 64*W[h, c*128+p]; fp8 main + fp8 32x-scaled residual
        WT = np.asarray(W, np.float32).T * WSCALE  # [D, H]
        wp = np.ascontiguousarray(WT.reshape(DCH, P, H).transpose(1, 0, 2))
        w8 = wp.astype(f8)
        ws8 = ((wp - w8.astype(np.float32)) * 32.0).astype(f8)
        return np.ascontiguousarray(w8), np.ascontiguousarray(ws8)

    wq8, wqs8 = wsplit(Wq)
    wk8, wks8 = wsplit(Wk)
    wv8, ws8 = wsplit(Wv)
    wpk = np.ascontiguousarray(
        np.stack([wk8, wq8, wks8, wqs8, wv8, ws8], axis=1)
    )
    c_ = 1.0 - LAMBDA_INIT
    rw = np.ascontiguousarray(
        np.asarray(rms_weight, np.float32) * c_ * math.sqrt(H)
    ).astype(np.float32)

    in_maps = []
    for core in range(NCORES):
        b, qb = divmod(core, NCORES // B)
        # own query block's columns first; key order is attention-irrelevant
        cols = np.r_[qb * QSHARD : (qb + 1) * QSHARD,
                     0 : qb * QSHARD, (qb + 1) * QSHARD : S]
        xc = xP[b][:, :, cols]
        x8 = np.ascontiguousarray(xc.astype(f8))
        r8 = np.ascontiguousarray(
            ((xc - x8.astype(np.float32)) * 32.0).astype(f8)
        )
        in_maps.append(
            {"x8": x8, "r8": r8, "wpk": wpk, "rmsw": rw}
        )
    # e4m3 and e4m3fn are byte-identical over our value range (<224); jax's
    # transfer path only accepts the fn variant, the device reads raw bytes
    fn = ml_dtypes.float8_e4m3fn
    for m in in_maps:
        for k, v in m.items():
            if v.dtype == f8:
                m[k] = v.view(fn)
    return in_maps


def kernel(x, Wq, Wk, Wv, lambda_q1, lambda_q2, lambda_k1, lambda_k2, rms_weight):
    lq1 = np.asarray(lambda_q1, np.float32)
    lq2 = np.asarray(lambda_q2, np.float32)
    lk1 = np.asarray(lambda_k1, np.float32)
    lk2 = np.asarray(lambda_k2, np.float32)
    lam = float(
        np.exp(np.dot(lq1, lk1)) - np.exp(np.dot(lq2, lk2)) + LAMBDA_INIT
    )
    nc = build(lam)
    in_maps = make_in_maps(x, Wq, Wk, Wv, rms_weight)
    res = bass_utils.run_bass_kernel_spmd(nc, in_maps, core_ids=list(range(NCORES)))
    out = np.empty((B, S, H), np.float32)
    for core in range(NCORES):
        b, qb = divmod(core, NCORES // B)
        out[b, qb * QSHARD : (qb + 1) * QSHARD] = res.results[core]["out"]
    return out


# revision 65
# speedup vs baseline: 1.0154x; 1.0000x over previous
"""DiffAttn (differential attention) Trainium2 Bass kernel — v3.

Self-contained: kernel(**inputs) takes the FULL unsharded inputs as numpy
arrays and returns the FULL output [2, 4096, 128] float32.

Sharding: 8 cores = (batch in {0,1}) x (query-block of 1024 rows). Each core
is FULLY independent (no collectives): it streams the whole batch's
activations (host-permuted to [p][c][q] with its own query block's columns
first), projects K and V for all 4096 keys locally, Q for its own 1024
queries, and runs both softmaxes + combined PV + RMSNorm for its queries.
Replicating the K/V projections beats the AllGather under the timeline model
(the collective is priced at ~67us, mostly unoverlappable) and keeps every
core's program trivially SPMD-identical.

fp8 projections (DoubleRow, 4x PE throughput): x and the weights are shipped
as e4m3 (weights pre-scaled by 64 so they sit in fp8's normal range; the 64^2
on the scores is folded into the softmax exp scale, exactly).  Q/K tolerate
plain fp8 (softmax output error ~0.1%).  V needs more precision, so V is
computed as  x8@wv8 + (x8@ws8 + r8@wv8)/32  where r8 = fp8(32*(x - x8)) and
ws8 = fp8(32*(wv' - wv8)) — a first-order residual expansion; the dropped
r*s cross term is ~0.1%.  The global 64x on V cancels in RMSNorm.

Attention layout: scores are computed TRANSPOSED ([sk, q], keys on
partitions) so exp(scores) feeds PV as the STATIONARY operand against a
MOVING V_aug = [V | ones] (129 cols); the ones column makes each PV
accumulation also produce the softmax denominator in column 128 of U — no
separate row-sum matmuls and no transposes in the post phase.  Key order is
attention-irrelevant, so K/V chunks process in host-permuted order.
Second-half heads live in base-partition-0 copies (kT2/qT2, moved with the
DVE stream-shuffle network): matmuls that share a PSUM bank must share a PE
row base (hardware tile-position rule), and the serial DMA queue is busy
streaming x when these copies are needed.

Post: attn = U1/s1 - lam*U2/s2; RMSNorm(attn) == RMSNorm(s1*attn), so
w := U1 - lam*(s1/s2)*U2 needs no 1/s1 division (torch eps is ~1e-5 of
mean(attn^2) here).  out = w * rsqrt(sum_h w^2) * [rmsw*(1-li)*sqrt(H)].
rsqrt runs on DVE (bit-trick + 2 Newton steps, ~5e-6 rel) so the ACT exp
stream never switches activation tables; the post runs on DVE/Pool per pass,
fully overlapped except the last pass's short chain.

PSUM (8 banks): s(2 bufs, 1 bank each) + 4 U slot banks + work(2 bufs) for
the next block's K/V accumulators.  Block 0's K/Q run c-inner in the four
U banks (idle until attention starts) so the PE ramps while x streams in.
"""

import math
import os
import sys
from contextlib import ExitStack

import numpy as np

for _p in ("/root/.axon_site/_ro/trn_rl_repo", "/opt/trn_rl_repo"):
    if os.path.isdir(_p) and _p not in sys.path:
        sys.path.append(_p)

import ml_dtypes  # noqa: E402

import concourse.bass as bass  # noqa: E402
import concourse.mybir as mybir  # noqa: E402
import concourse.tile as tile  # noqa: E402
from concourse import bacc, bass_utils  # noqa: E402

B, S, D, H = 2, 4096, 2048, 128
H2 = H // 2  # 64
P = 128
NCORES = 8
QSHARD = 1024  # q rows per core
DCH = D // P  # 16 d-chunks
NKCH = S // P  # 32 key chunks of 128
NBLK, BLKW = 4, 1024  # key blocks (1024 keys each)
NPASS, PW = 4, 256  # query passes of 256 columns (U psum = 4 banks)
WSCALE = 64.0  # host-side weight prescale (exact power of 2)

LAMBDA_INIT = 0.8 - 0.6 * math.exp(-0.3 * 12)
SCALE = 1.0 / math.sqrt(H2)
SCALE_EXP = SCALE / (WSCALE * WSCALE)  # exp input is (64q).(64k)

F32 = mybir.dt.float32
BF16 = mybir.dt.bfloat16
F8 = mybir.dt.float8e4
I32 = mybir.dt.int32

AF = mybir.ActivationFunctionType
OP = mybir.AluOpType
DR = mybir.MatmulPerfMode.DoubleRow


def _emit(ctx: ExitStack, tc: "tile.TileContext", lam: float):  # noqa: C901
    nc = tc.nc

    x8_d = nc.dram_tensor("x8", (P, DCH, S), F8, kind="ExternalInput").ap()
    r8_d = nc.dram_tensor("r8", (P, DCH, S), F8, kind="ExternalInput").ap()
    wpk_d = nc.dram_tensor("wpk", (P, 6, DCH, H), F8, kind="ExternalInput").ap()
    rmsw = nc.dram_tensor("rmsw", (H,), F32, kind="ExternalInput").ap()
    out_d = nc.dram_tensor("out", (QSHARD, H), F32, kind="ExternalOutput").ap()

    # ---- persistent SBUF ----
    consts = ctx.enter_context(tc.tile_pool(name="consts", bufs=1))
    persist = ctx.enter_context(tc.tile_pool(name="persist", bufs=1))

    wpk_sb = consts.tile([P, 6, DCH, H], F8)
    wk_sb, wq_sb, wks_sb, wqs_sb, wv_sb, ws_sb = (
        wpk_sb[:, i] for i in range(6)
    )
    rmsw_bc = consts.tile([P, H], F32)

    kT_sb = persist.tile([P, S], BF16)  # [h, key]
    qT_sb = persist.tile([P, QSHARD], BF16)  # [h, q]
    # second-half heads relocated to base partition 0 (PE tile-position rule)
    kT2_sb = persist.tile([64, S], BF16)
    qT2_sb = persist.tile([64, QSHARD], BF16)
    v_sb = persist.tile([P, NKCH, H + 1], BF16)  # [sk%128, chunk, h|1]
    u_acc = persist.tile([P, NPASS, 2, 2, H + 1], F32)  # [q, pass, hf, j2, h|s]
    w_all = persist.tile([P, 2 * NPASS, H], F32)  # [q, subtile, h]
    rsq_all = persist.tile([P, 2 * NPASS], F32)
    rr_all = persist.tile([P, 2 * NPASS], F32)
    rr_i = persist.tile([P, 2 * NPASS], I32)
    o_all = persist.tile([P, 2 * NPASS, H], F32)

    xpool = ctx.enter_context(tc.tile_pool(name="xstream", bufs=1))
    epool = ctx.enter_context(tc.tile_pool(name="epool", bufs=46))
    small = ctx.enter_context(tc.tile_pool(name="small", bufs=8))

    # ---- const DMAs; ones column of V_aug.  Weights ship as ONE packed
    # tensor (per-partition rows are 12KB contiguous: no small-descriptor
    # penalty); the c0-1 slice of the four k/q weights goes first ----
    nc.sync.dma_start(out=wpk_sb[:, 0:4, 0:2, :], in_=wpk_d[:, 0:4, 0:2, :])
    nc.vector.memset(v_sb[:, :, H : H + 1], 1.0)

    x_tiles, r_tiles = [], []
    for b in range(NBLK):
        x_tiles.append(
            xpool.tile([P, DCH, BLKW], F8, tag=f"x{b % 3}", bufs=1, name=f"xt{b}")
        )
        r_tiles.append(
            xpool.tile([P, DCH, BLKW], F8, tag=f"r{b % 3}", bufs=1, name=f"rt{b}")
        )
    # block-0 x8/r8 interleaved in c-pair slabs: the residual projections
    # pipeline with the DMA instead of waiting for the whole r8 block
    for cp in range(DCH // 2):
        nc.sync.dma_start(
            out=x_tiles[0][:, 2 * cp : 2 * cp + 2, :],
            in_=x8_d[:, 2 * cp : 2 * cp + 2, 0:BLKW],
        )
        if cp == 0:
            nc.sync.dma_start(
                out=wpk_sb[:, 0:4, 2:DCH, :], in_=wpk_d[:, 0:4, 2:DCH, :]
            )
        if cp == 1:
            nc.sync.dma_start(out=wpk_sb[:, 4:6], in_=wpk_d[:, 4:6])
        if cp == 2:
            nc.sync.dma_start(
                out=rmsw_bc,
                in_=bass.AP(tensor=rmsw.tensor, offset=0, ap=[[0, P], [1, H]]),
            )
        nc.sync.dma_start(
            out=r_tiles[0][:, 2 * cp : 2 * cp + 2, :],
            in_=r8_d[:, 2 * cp : 2 * cp + 2, 0:BLKW],
        )

    def x_slabs(b):
        for h2 in range(2):
            nc.sync.dma_start(
                out=x_tiles[b][:, h2 * 8 : (h2 + 1) * 8, :],
                in_=x8_d[:, h2 * 8 : (h2 + 1) * 8, b * BLKW : (b + 1) * BLKW],
            )
            nc.sync.dma_start(
                out=r_tiles[b][:, h2 * 8 : (h2 + 1) * 8, :],
                in_=r8_d[:, h2 * 8 : (h2 + 1) * 8, b * BLKW : (b + 1) * BLKW],
            )

    x_slabs(1)

    psum = ctx.enter_context(tc.tile_pool(name="psum", space="PSUM", bufs=1))

    def proj_copy_k(accm, accr, b, g):
        # hw: ALU ops may read only ONE input from PSUM -> copy main first,
        # then accumulate the scaled residual in place.  High priority: this
        # DVE chain gates the next block's entire exp stream.
        sl = slice(b * BLKW + g * 512, b * BLKW + (g + 1) * 512)
        with tc.high_priority(offset=1500):
            nc.vector.tensor_copy(kT_sb[:, sl], accm)
            nc.vector.scalar_tensor_tensor(
                kT_sb[:, sl], accr, 1.0 / 32.0, kT_sb[:, sl],
                op0=OP.mult, op1=OP.add,
            )
        # second-half heads to base partition 0 via the DVE shuffle network
        # (no DMA: the serial DMA queue is full of x slabs at this point)
        nc.vector.stream_shuffle(kT2_sb[:, sl], kT_sb[64:128, sl], list(range(32)))

    def kq_mm(acc, w_t, xt, cp, g, st, sp):
        nc.tensor.matmul(
            acc,
            w_t[:, 2 * cp : 2 * cp + 2, :],
            xt[:, 2 * cp : 2 * cp + 2, g * 512 : (g + 1) * 512],
            start=st,
            stop=sp,
            perf_mode=DR,
        )

    # ---- block 0: K and Q projections c-inner in the four (idle) U banks,
    # pipelined with the x8 c-pair DMAs so the PE starts early and ramps ----
    kaccm = [
        psum.tile([P, 512], F32, tag=f"u{g}", bufs=1, name=f"kaccm{g}")
        for g in range(2)
    ]
    qaccm = [
        psum.tile([P, 512], F32, tag=f"u{2 + g}", bufs=1, name=f"qaccm{g}")
        for g in range(2)
    ]
    kaccr = [
        psum.tile([P, 512], F32, tag="s", bufs=2, name=f"kaccr{g}")
        for g in range(2)
    ]
    qaccr = [
        psum.tile([P, 512], F32, tag="work", bufs=2, name=f"qaccr{g}")
        for g in range(2)
    ]
    # per c-pair: main, x8*ws residual, r8*w residual — everything only needs
    # that c-pair of x8/r8, so the whole phase is DMA-paced
    for cp in range(DCH // 2):
        st, sp = cp == 0, cp == DCH // 2 - 1
        for g in range(2):
            kq_mm(kaccm[g], wk_sb, x_tiles[0], cp, g, st, sp)
            kq_mm(qaccm[g], wq_sb, x_tiles[0], cp, g, st, sp)
            kq_mm(kaccr[g], wks_sb, x_tiles[0], cp, g, st, False)
            kq_mm(qaccr[g], wqs_sb, x_tiles[0], cp, g, st, False)
            kq_mm(kaccr[g], wk_sb, r_tiles[0], cp, g, False, sp)
            kq_mm(qaccr[g], wq_sb, r_tiles[0], cp, g, False, sp)
    for g in range(2):
        proj_copy_k(kaccm[g], kaccr[g], 0, g)
        qsl = slice(g * 512, (g + 1) * 512)
        nc.scalar.copy(qT_sb[:, qsl], qaccm[g])
        nc.vector.scalar_tensor_tensor(
            qT_sb[:, qsl], qaccr[g], 1.0 / 32.0, qT_sb[:, qsl],
            op0=OP.mult, op1=OP.add,
        )
        nc.vector.stream_shuffle(
            qT2_sb[:, g * 512 : (g + 1) * 512],
            qT_sb[64:128, g * 512 : (g + 1) * 512],
            list(range(32)),
        )
    x_slabs(2)
    x_slabs(3)

    def kproj_gen(b):
        """K projection for block b (fp8 main + residual, two work banks),
        sliced: yields between groups so emission interleaves with the
        attention chunk stream (the PE queue drains in order)."""
        for g in range(2):
            accm = psum.tile([P, 512], F32, tag="work", bufs=2, name=f"km{b}{g}")
            accr = psum.tile([P, 512], F32, tag="work", bufs=2, name=f"kr{b}{g}")
            for cp in range(DCH // 2):
                kq_mm(accm, wk_sb, x_tiles[b], cp, g, cp == 0, cp == DCH // 2 - 1)
            yield
            for cp in range(DCH // 2):
                kq_mm(accr, wks_sb, x_tiles[b], cp, g, cp == 0, False)
            for cp in range(DCH // 2):
                kq_mm(accr, wk_sb, r_tiles[b], cp, g, False, cp == DCH // 2 - 1)
            proj_copy_k(accm, accr, b, g)
            yield

    def vproj_gen(b):
        """V projection for block b: fp8 main term plus (1/32)-scaled
        residual terms, two work banks, j-outer (sequential groups/bank);
        yields between j-slices for emission interleaving."""
        for g in range(2):
            vmain = psum.tile([P, 4, P], F32, tag="work", bufs=2, name=f"vm{b}{g}")
            vres = psum.tile([P, 4, P], F32, tag="work", bufs=2, name=f"vr{b}{g}")
            for j in range(4):
                ksl = slice(g * 512 + j * P, g * 512 + (j + 1) * P)
                for cp in range(DCH // 2):
                    csl = slice(2 * cp, 2 * cp + 2)
                    nc.tensor.matmul(
                        vmain[:, j, :],
                        x_tiles[b][:, csl, ksl],
                        wv_sb[:, csl, :],
                        start=(cp == 0),
                        stop=(cp == DCH // 2 - 1),
                        perf_mode=DR,
                    )
                for cp in range(DCH // 2):
                    csl = slice(2 * cp, 2 * cp + 2)
                    nc.tensor.matmul(
                        vres[:, j, :],
                        x_tiles[b][:, csl, ksl],
                        ws_sb[:, csl, :],
                        start=(cp == 0),
                        stop=False,
                        perf_mode=DR,
                    )
                for cp in range(DCH // 2):
                    csl = slice(2 * cp, 2 * cp + 2)
                    nc.tensor.matmul(
                        vres[:, j, :],
                        r_tiles[b][:, csl, ksl],
                        wv_sb[:, csl, :],
                        start=False,
                        stop=(cp == DCH // 2 - 1),
                        perf_mode=DR,
                    )
                yield
            vsl = v_sb[:, b * 8 + g * 4 : b * 8 + (g + 1) * 4, 0:H]
            nc.vector.tensor_copy(vsl, vmain)
            nc.vector.scalar_tensor_tensor(
                vsl, vres, 1.0 / 32.0, vsl, op0=OP.mult, op1=OP.add
            )

    def run_all(gen):
        for _ in gen:
            pass

    stage = int(os.environ.get("KSTAGE", "3"))
    if stage <= 1:
        run_all(vproj_gen(0))
        for b in range(1, NBLK):
            run_all(kproj_gen(b))
            run_all(vproj_gen(b))
        o_dbg = small.tile([P, H], F32, tag="odbg")
        nc.vector.tensor_copy(o_dbg, kT_sb[:, 0:H])
        for t in range(2 * NPASS):
            nc.sync.dma_start(out=out_d[t * P : (t + 1) * P, :], in_=o_dbg)
        return

    def attend_block_pass(b, p, fillers=iter(())):
        """Pass p (256 q cols), all 8 key chunks of block b -> U psum, then
        drain-add into u_acc.  A filler slice of the next block's projection
        work is emitted mid-pass and at pass end."""
        q0 = p * PW
        u_ps = [
            psum.tile([P, 512], F32, tag=f"u{s}", bufs=1, name=f"u{b}_{p}_{s}")
            for s in range(4)
        ]
        for cl in range(8):
            ch = b * 8 + cl
            s_ps = psum.tile([P, 512], F32, tag="s", bufs=2)
            with tc.high_priority(offset=5000):
                nc.tensor.matmul(
                    s_ps[:, 0:PW],
                    kT_sb[0:H2, ch * P : (ch + 1) * P],
                    qT_sb[0:H2, q0 : q0 + PW],
                )
                nc.tensor.matmul(
                    s_ps[:, PW : 2 * PW],
                    kT2_sb[:, ch * P : (ch + 1) * P],
                    qT2_sb[:, q0 : q0 + PW],
                )
                e_sb = epool.tile([P, 512], BF16, tag="e")
                nc.scalar.activation(e_sb, s_ps, AF.Exp, scale=SCALE_EXP)
            for hf in range(2):
                for j2 in range(2):
                    nc.tensor.matmul(
                        u_ps[hf * 2 + j2][:, 0 : H + 1],
                        e_sb[:, hf * PW + j2 * P : hf * PW + (j2 + 1) * P],
                        v_sb[:, ch, :],
                        start=(cl == 0),
                        stop=(cl == 7),
                    )
        with tc.high_priority(offset=600):
            for slot in range(4):
                hf, j2 = divmod(slot, 2)
                dst = u_acc[:, p, hf, j2, :]
                src = u_ps[slot][:, 0 : H + 1]
                if b == 0:
                    nc.vector.tensor_copy(dst, src)
                else:
                    nc.vector.tensor_tensor(dst, src, dst, op=OP.add)

    def post_pass(p):
        """Normalize/combine for pass p's two q-subtiles (DVE + Pool only)."""
        for j2 in range(2):
            t = p * 2 + j2
            s1 = u_acc[:, p, 0, j2, H : H + 1]
            s2 = u_acc[:, p, 1, j2, H : H + 1]
            r2 = small.tile([P, 1], F32, tag="r2")
            nc.vector.reciprocal(r2, s2)
            al = small.tile([P, 1], F32, tag="al")
            nc.vector.scalar_tensor_tensor(
                al, s1, -lam, r2, op0=OP.mult, op1=OP.mult
            )
            nc.vector.scalar_tensor_tensor(
                w_all[:, t, :],
                u_acc[:, p, 1, j2, 0:H],
                al,
                u_acc[:, p, 0, j2, 0:H],
                op0=OP.mult,
                op1=OP.add,
            )
            sq = small.tile([P, H], F32, tag="sq")
            nc.gpsimd.tensor_tensor(sq, w_all[:, t, :], w_all[:, t, :], op=OP.mult)
            nc.vector.tensor_reduce(
                rsq_all[:, t : t + 1], sq, axis=mybir.AxisListType.X, op=OP.add
            )
        # rr = rsqrt(ssq) on DVE (bit-trick + 2 Newton steps, ~5e-6 rel):
        # keeps the ACT exp stream free of Sqrt table switches
        t0 = p * 2
        ss = rsq_all[:, t0 : t0 + 2]
        yi = rr_i[:, t0 : t0 + 2]
        y = yi.bitcast(F32)
        nc.vector.tensor_scalar(
            yi, ss.bitcast(I32), 1, None, op0=OP.arith_shift_right
        )
        nc.vector.tensor_scalar(yi, yi, -1, 0x5F3759DF, op0=OP.mult, op1=OP.add)
        nt = small.tile([P, 2], F32, tag="nt")
        for _ in range(2):
            nc.vector.tensor_tensor(nt, ss, y, op=OP.mult)
            nc.vector.tensor_tensor(nt, nt, y, op=OP.mult)
            nc.vector.tensor_scalar(nt, nt, -0.5, 1.5, op0=OP.mult, op1=OP.add)
            nc.vector.tensor_tensor(y, y, nt, op=OP.mult)
        nc.vector.tensor_copy(rr_all[:, t0 : t0 + 2], y)
        for t in (t0, t0 + 1):
            nc.vector.scalar_tensor_tensor(
                o_all[:, t, :], w_all[:, t, :], rr_all[:, t : t + 1], rmsw_bc,
                op0=OP.mult, op1=OP.mult,
            )
        nc.scalar.dma_start(
            out=out_d[t0 * P : (t0 + 2) * P, :].rearrange("(t p) h -> p t h", p=P),
            in_=o_all[:, t0 : t0 + 2, :],
        )

    # ---- main schedule: per block: 4 passes, with the next block's K/V
    # projection work sliced into small fillers emitted between chunks so
    # the PE queue interleaves them with the score/exp stream ----
    run_all(vproj_gen(0))
    for b in range(NBLK):
        for p in range(NPASS):
            attend_block_pass(b, p)
            if b < NBLK - 1:
                if p == 1:
                    run_all(kproj_gen(b + 1))
                elif p == 2:
                    vg = vproj_gen(b + 1)
                    for _ in range(4):
                        next(vg, None)
                elif p == 3:
                    for _ in vg:
                        pass
            if b == NBLK - 1 and stage >= 3:
                with tc.high_priority(offset=900):
                    post_pass(p)


def build(lam: float):
    from concourse._compat import axon_active

    nc = bacc.Bacc(
        "TRN2",
        target_bir_lowering=False,
        debug=not axon_active(),
        num_devices=NCORES,
    )
    with tile.TileContext(nc) as tc:
        with ExitStack() as ctx:
            _emit(ctx, tc, lam)
    nc.compile()
    return nc


def make_in_maps(x, Wq, Wk, Wv, rms_weight):
    f8 = ml_dtypes.float8_e4m3
    x = np.asarray(x, dtype=np.float32)
    xT = np.ascontiguousarray(x.transpose(0, 2, 1))  # [B, D, S]
    xP = xT.reshape(B, DCH, P, S).transpose(0, 2, 1, 3)  # [B, p, c, S]

    def wsplit(W):
        # w'[p, c, h] = 64*W[h, c*128+p]; fp8 main + fp8 32x-scaled residual
        WT = np.asarray(W, np.float32).T * WSCALE  # [D, H]
        wp = np.ascontiguousarray(WT.reshape(DCH, P, H).transpose(1, 0, 2))
        w8 = wp.astype(f8)
        ws8 = ((wp - w8.astype(np.float32)) * 32.0).astype(f8)
        return np.ascontiguousarray(w8), np.ascontiguousarray(ws8)

    wq8, wqs8 = wsplit(Wq)
    wk8, wks8 = wsplit(Wk)
    wv8, ws8 = wsplit(Wv)
    wpk = np.ascontiguousarray(
        np.stack([wk8, wq8, wks8, wqs8, wv8, ws8], axis=1)
    )
    c_ = 1.0 - LAMBDA_INIT
    rw = np.ascontiguousarray(
        np.asarray(rms_weight, np.float32) * c_ * math.sqrt(H)
    ).astype(np.float32)

    in_maps = []
    for core in range(NCORES):
        b, qb = divmod(core, NCORES // B)
        # own query block's columns first; key order is attention-irrelevant
        cols = np.r_[qb * QSHARD : (qb + 1) * QSHARD,
                     0 : qb * QSHARD, (qb + 1) * QSHARD : S]
        xc = xP[b][:, :, cols]
        x8 = np.ascontiguousarray(xc.astype(f8))
        r8 = np.ascontiguousarray(
            ((xc - x8.astype(np.float32)) * 32.0).astype(f8)
        )
        in_maps.append(
            {"x8": x8, "r8": r8, "wpk": wpk, "rmsw": rw}
        )
    # e4m3 and e4m3fn are byte-identical over our value range (<224); jax's
    # transfer path only accepts the fn variant, the device reads raw bytes
    fn = ml_dtypes.float8_e4m3fn
    for m in in_maps:
        for k, v in m.items():
            if v.dtype == f8:
                m[k] = v.view(fn)
    return in_maps


def kernel(x, Wq, Wk, Wv, lambda_q1, lambda_q2, lambda_k1, lambda_k2, rms_weight):
    lq1 = np.asarray(lambda_q1, np.float32)
    lq2 = np.asarray(lambda_q2, np.float32)
    lk1 = np.asarray(lambda_k1, np.float32)
    lk2 = np.asarray(lambda_k2, np.float32)
    lam = float(
        np.exp(np.dot(lq1, lk1)) - np.exp(np.dot(lq2, lk2)) + LAMBDA_INIT
    )
    nc = build(lam)
    in_maps = make_in_maps(x, Wq, Wk, Wv, rms_weight)
    res = bass_utils.run_bass_kernel_spmd(nc, in_maps, core_ids=list(range(NCORES)))
    out = np.empty((B, S, H), np.float32)
    for core in range(NCORES):
        b, qb = divmod(core, NCORES // B)
        out[b, qb * QSHARD : (qb + 1) * QSHARD] = res.results[core]["out"]
    return out
